# revision 49
# baseline (speedup 1.0000x reference)
"""CorrRatio (symmetric correlation-ratio loss) on 8 Trainium2 NeuronCores.

Strategy
--------
Input: y_true, y_pred f32 (1,1,128,128,128) -> N = 2^21 voxels, sharded
contiguously across 8 cores as [128, 2048] f32 tiles (all reductions are
order-independent, so contiguous sharding is exact).

Quantiles are computed EXACTLY on the host (np.quantile) - no device
phase needed. One NEFF does the whole main pass:

Per direction (A: target=y_pred/x=y_true, B: swapped), bins live on the
normalized axis yn = (clip(t)-lo)*ivf in [0,32]:
 * 8 ANCHOR bins a=0,4,..,28: ACT Derivative_Erf(scale*tc+bias) -> w_a
   (bf16), with accum_out giving S0_a per-partition for free.
 * 4 E tiles (one per anchor PAIR ae=0,8,16,24): ACT Exp. E encodes
   exp(2PT*(yn-ae-.5)); pairs ae=0,8 get a DVE min-clamp (ACT Exp
   overflows to Inf above arg 88, measured). Second zone of each pair
   uses E2 = E*exp(-2PT*4) (one DVE tensor_scalar per pair) so chain
   ops stay plain 2x-mode tensor_tensor.
 * 24 CHAIN bins: w_{a+j} = w_{a+j-1} * E(2) on DVE bf16 (j<=3;
   unnormalized - host multiplies gamma_j = exp(-PT j^2) back).
 * S0 for chain bins + SX/SX2: one-hot PE folds, col-tiled 4-way
   (tile_position via out base partition 0/32/64/96).
 * S1 for ALL 32 bins: PE "trace" matmuls - lhsT = 64-col chunk of
   x (bf16), rhs = the 8 bins of a group x same 64 cols, accumulated
   over 32 chunks into PSUM [64,8,64]; the diagonal [i,(b,i)] holds
   sum(w_b * x) partials. Host extracts diagonals (f64). This removes
   all 64 per-bin w*x DVE multiplies.

Everything (PSUM S1/S0 partials, ACT accums) is DMA'd out raw; the host
does the small algebra in f64.
"""

import numpy as np

import concourse.bacc as bacc
import concourse.bass as bass
import concourse.mybir as mybir
import concourse.tile as tile
from concourse import bass_utils

F32 = mybir.dt.float32
BF16 = mybir.dt.bfloat16
ALU = mybir.AluOpType
ACTF = mybir.ActivationFunctionType

NB = 32
SR = 1.0 / 2.355
PT = 1.0 / (2.0 * SR * SR)          # normalized preterm = 2.355^2/2
SQPT = float(np.sqrt(PT))
EPS = float(np.finfo(np.float32).eps)
NCORES = 8
N = 128 * 128 * 128                  # 2097152 voxels
V = N // NCORES                      # 262144 per core
P = 128
F = V // P                           # 2048 free-dim per partition

ZONE = 4                             # bins per anchor
NA = NB // ZONE                      # 8 anchors/dir: 0,4,...,28
NPAIR = NA // 2                      # 4 E tiles/dir (anchor pairs)
C_PAIR = float(np.float32(np.exp(-2.0 * PT * ZONE)))   # E2 = E * C_PAIR
ANCHORS = [ZONE * i for i in range(NA)]
CHAIN_BINS = [8 * g + o for g in range(4) for o in (1, 2, 3, 5, 6, 7)]
ECLAMP = 3.0e38                      # min-clamp for Exp Inf overflow

CONV_Z = ()                # zones whose slot-2 bin is a
                                     # direct DErf (both dirs): ACT has
                                     # slack, DVE is the wall
NCC = 40                             # cst columns


def _build_main():
    nc = bacc.Bacc("TRN2", target_bir_lowering=False, debug=False,
                   num_devices=NCORES)
    yt_d = nc.dram_tensor("yt", [P, F], F32, kind="ExternalInput").ap()
    yp_d = nc.dram_tensor("yp", [P, F], F32, kind="ExternalInput").ap()
    cst_d = nc.dram_tensor("cst", [P, NCC], F32, kind="ExternalInput").ap()
    s1_d = [nc.dram_tensor(f"s1_{d}", [P, 4, 4, 64], F32,
                           kind="ExternalOutput").ap() for d in range(2)]
    s0_d = [nc.dram_tensor(f"s0_{d}", [P, 512], F32,
                           kind="ExternalOutput").ap() for d in range(2)]
    acc_d = nc.dram_tensor("acc", [P, 32], F32, kind="ExternalOutput").ap()

    with tile.TileContext(nc) as tc:
        with (
            tc.tile_pool(name="io", bufs=1) as io_pool,
            tc.tile_pool(name="clip", bufs=1) as clip_pool,
            tc.tile_pool(name="xp", bufs=1) as x_pool,
            tc.tile_pool(name="ep", bufs=3) as e_pool,
            tc.tile_pool(name="wp", bufs=2) as w_pool,
            tc.tile_pool(name="stat", bufs=1) as stat_pool,
            tc.tile_pool(name="psum", bufs=1, space="PSUM") as psum_pool,
        ):
            yt = io_pool.tile([P, F], F32)
            yp = io_pool.tile([P, F], F32)
            cst = io_pool.tile([P, NCC], F32)
            FH = F // 2
            nc.sync.dma_start(cst[:], cst_d)
            nc.sync.dma_start(yp[:], yp_d)
            nc.sync.dma_start(yt[:], yt_d)

            # Dummy 1-element Exp on cst: pulls the ~1.5us Exp table load
            # off the critical path (it otherwise waits for the first real
            # Exp's input clip at ~9us).
            warm = stat_pool.tile([P, 1], BF16, tag="warm", name="warm")
            nc.scalar.activation(warm[:], cst[:, 0:1], ACTF.Exp)

            # --- clips, IN-PLACE (elementwise same-index streaming is
            # read-ahead-of-write through the DVE pipe): frees 16KB/part,
            # spent on a deeper wz pool
            nc.vector.tensor_scalar(
                out=yp[:], in0=yp[:], scalar1=cst[:, 31:32],
                scalar2=cst[:, 30:31], op0=ALU.min, op1=ALU.max)
            nc.vector.tensor_scalar(
                out=yt[:], in0=yt[:], scalar1=cst[:, 29:30],
                scalar2=cst[:, 28:29], op0=ALU.min, op1=ALU.max)
            ypc, ytc = yp, yt
            xb = []                       # xb[d]: the "x" tensor of dir d
            for d, src in ((0, ytc), (1, ypc)):
                t = x_pool.tile([P, F], BF16, tag=f"xb{d}", name=f"xb{d}")
                nc.vector.tensor_scalar(out=t[:], in0=src[:], scalar1=1.0,
                                        scalar2=None, op0=ALU.mult)
                xb.append(t)
            xsq = []
            for d in range(2):
                t = x_pool.tile([P, F], BF16, tag=f"xsq{d}", name=f"xsq{d}")
                nc.scalar.activation(t[:], xb[d][:], ACTF.Square)
                xsq.append(t)

            # pre-scaled bin-domain inputs, IN-PLACE on the clipped tiles
            # (the xb casts above already consumed the unscaled values;
            # WAR is serialized by the framework). ACT AP-scale measures
            # ~0.5us extra per op, so scale once on DVE; DErf uses scale=1
            # and Exp the compile-time 2*SQPT.
            for d, src_t in ((0, ypc), (1, ytc)):
                nc.vector.tensor_scalar(out=src_t[:], in0=src_t[:],
                                        scalar1=cst[:, 24 + d:25 + d],
                                        scalar2=None, op0=ALU.mult)
            tcs = (ypc, ytc)              # target (binned) tensor per dir

            # --- sliding one-hot for S0 folds: Z[:, 31] = 1
            Z = stat_pool.tile([P, 63], BF16)
            nc.vector.memset(Z[:], 0.0)
            nc.vector.memset(Z[:, 31:32], 1.0)

            # --- accum tile for anchor S0 (ACT accum_out)
            acc = stat_pool.tile([P, 32], F32)

            # --- w-tile allocation + anchor DErf (hoistable so the first
            # zone-pair's anchors precede the Exp block: costs one extra
            # ACT table switch but removes ~15us of pipeline head).
            wz_tiles = {}

            def mk_tiles(q):
                pair = []
                for z in (2 * q, 2 * q + 1):
                    wz = w_pool.tile([P, 2, 4, F], BF16, tag="wz", bufs=3,
                                     name=f"w_{z}")
                    for d in range(2):
                        nc.scalar.activation(
                            wz[:, d, 0, :], tcs[d][:], ACTF.Derivative_Erf,
                            bias=cst[:, 8 * d + z:8 * d + z + 1],
                            scale=1.0,
                            accum_out=acc[:, 8 * d + z:8 * d + z + 1])
                    pair.append(wz)
                wz_tiles[q] = pair

            # --- E tiles. ACT order minimizes table switches while
            # letting the pipeline start early: Exp(pair0) -> DErf(q0)
            # -> Exp(pairs 1-3) -> DErf(q1..q3). The DVE parts (Inf-clamp
            # for pairs 0/1, E2 = E*C_PAIR) are deferred into the q-loop
            # so chains of earlier zones aren't queued behind them.
            eraw = {}

            def mk_exp(pi):
                t = e_pool.tile([P, 2, F], BF16, tag="eraw", bufs=4,
                                name=f"eraw_{pi}")
                for d in range(2):
                    nc.scalar.activation(
                        t[:, d, :], tcs[d][:], ACTF.Exp,
                        bias=cst[:, 16 + 4 * d + pi:17 + 4 * d + pi],
                        scale=2.0 * SQPT)
                eraw[pi] = t

            e1 = {}
            e2 = {}

            def mk_epost(pi):
                r = eraw[pi]
                if pi < 2:                # ae = 0, 8: Exp can hit Inf
                    t = e_pool.tile([P, 2, F], BF16, tag="ecl", bufs=2,
                                    name=f"ecl_{pi}")
                    nc.vector.tensor_scalar(out=t[:], in0=r[:],
                                            scalar1=ECLAMP, scalar2=None,
                                            op0=ALU.min)
                    e1[pi] = t
                else:
                    e1[pi] = r
                t2 = e_pool.tile([P, 2, F], BF16, tag="e2", bufs=2,
                                 name=f"e2_{pi}")
                nc.vector.tensor_scalar(out=t2[:], in0=e1[pi][:],
                                        scalar1=C_PAIR, scalar2=None,
                                        op0=ALU.mult)
                e2[pi] = t2

            mk_exp(0)
            mk_tiles(0)
            for pi in (1, 2, 3):
                mk_exp(pi)

            # --- PSUM tiles
            # ps1[d][64*(z%2)+i, z//2, b, i'] accumulates
            #   sum_c sum_p xb[p,64c+i] * w_{4z+b}[p,64c+i']  (diag i==i')
            ps1 = [psum_pool.tile([P, 4, 4, 64], F32, tag=f"ps1_{d}",
                                  name=f"ps1_{d}") for d in range(2)]
            ps0 = [psum_pool.tile([P, 512], F32, tag=f"ps0_{d}",
                                  name=f"ps0_{d}") for d in range(2)]


            # S0 fold bookkeeping: chain bin k=4z+o -> col-group j=z%4,
            # row 3*(z//4)+(o-1); SX -> (j=0,row 6), SX2 -> (j=1,row 6).
            # One PSUM accumulation group per (d, col-group): count MMs so
            # start/stop land on the first/last.
            nmm_j = []                    # per col-group MM counts
            for j in range(4):
                n = 4 if j < 2 else 0     # SX / SX2
                for z in (j, j + 4):
                    n += 4 * sum(1 for o in (1, 2, 3)
                                 if not (o == 2 and z in CONV_Z))
                nmm_j.append(n)
            mm_ctr = [[0] * 4 for _ in range(2)]

            def s0_fold_mm(d, j, row, rhs_tile, slot, c):
                rhs = (rhs_tile[:, slot[0], slot[1], c * FC:(c + 1) * FC]
                       if slot is not None
                       else rhs_tile[:, c * FC:(c + 1) * FC])
                i = mm_ctr[d][j]
                nc.tensor.matmul(
                    ps0[d][32 * j:32 * j + 32, :],
                    Z[:, 31 - row:63 - row], rhs,
                    start=(i == 0), stop=(i == nmm_j[j] - 1),
                    tile_position=(0, 32 * j))
                mm_ctr[d][j] += 1

            # --- per direction, per zone PAIR (2 zones of anchor+3 chains).
            # Matmuls of the two zones interleave so adjacent MMs hit
            # disjoint PE col-groups (out base 0-63 vs 64-127 / col-group
            # j vs j+1) and stream concurrently.
            FC = F // 4                   # 512-col chunks for S0 folds
            CH = 64                       # 64-col chunks for S1 traces
            NCH = F // CH                 # 32
            # SX / SX2 folds first: their inputs are ready early, and
            # putting them first in the (d, col-group) accumulation groups
            # moves them off the kernel tail.
            for c in range(4):
                for d in range(2):
                    s0_fold_mm(d, 0, 6, xb[d], None, c)
                    s0_fold_mm(d, 1, 6, xsq[d], None, c)

            for q in range(4):
                mk_epost(q)
                # prefetch next pair's anchor DErfs on ACT so its chains
                # (DVE) aren't serialized behind this pair's PE/ACT work
                if q + 1 < 4:
                    mk_tiles(q + 1)
                if q not in wz_tiles:
                    mk_tiles(q)
                wzs = wz_tiles[q]
                # chains, dir-merged: one [P, 2, F] TT per (zone, slot)
                for zi, z in enumerate((2 * q, 2 * q + 1)):
                    et = e1[q] if z % 2 == 0 else e2[q]
                    for slot in (1, 2, 3):
                        nc.vector.tensor_tensor(
                            out=wzs[zi][:, :, slot, :],
                            in0=wzs[zi][:, :, slot - 1, :],
                            in1=et[:], op=ALU.mult)

                for d in range(2):
                    # S1 trace matmuls, pair-interleaved over chunks
                    for c in range(NCH):
                        for zi, z in enumerate((2 * q, 2 * q + 1)):
                            nc.tensor.matmul(
                                ps1[d][64 * zi:64 * zi + 64, z // 2, :, :],
                                xb[d][:, c * CH:(c + 1) * CH],
                                wzs[zi][:, d, :, c * CH:(c + 1) * CH],
                                start=(c == 0), stop=(c == NCH - 1))

                    # S0 one-hot folds, pair-interleaved (adjacent j's)
                    for o in (1, 2, 3):
                        for c in range(4):
                            for zi, z in enumerate((2 * q, 2 * q + 1)):
                                s0_fold_mm(d, z % 4, 3 * (z // 4) + (o - 1),
                                           wzs[zi], (d, o), c)

                    # stage+DMA this pair's finished S1 PSUM slice (PSUM is
                    # not DMA-able; ACT Copy - it's in every table set and
                    # ScalarE sits closest to PSUM). Overlaps later pairs.
                    st = stat_pool.tile([P, 4, 64], F32, tag=f"s1s{d}",
                                        bufs=2, name=f"s1s{d}_{q}")
                    nc.scalar.activation(st[:], ps1[d][:, q, :, :], ACTF.Copy)
                    nc.sync.dma_start(s1_d[d][:, q, :, :], st[:])

            # --- DMA the rest out (host does the algebra)
            nc.sync.dma_start(acc_d, acc[:])
            for d in range(2):
                s0s = stat_pool.tile([P, 512], F32, tag=f"s0s{d}",
                                     name=f"s0s{d}")
                nc.scalar.activation(s0s[:], ps0[d][:], ACTF.Copy)
                nc.sync.dma_start(s0_d[d], s0s[:])
    nc.compile()
    return nc


_NC_CACHE = {}


def _get_nc(which="main"):
    if which not in _NC_CACHE:
        _NC_CACHE[which] = _build_main()
    return _NC_CACHE[which]


def _run(nc, in_maps, trace=False):
    return bass_utils.run_bass_kernel_spmd(
        nc, in_maps, core_ids=list(range(NCORES)), trace=trace)


def _cst(qyt_lo, qyt_hi, qyp_lo, qyp_hi):
    row = np.zeros(NCC, dtype=np.float32)
    # dir 0 (A): target=yp, x=yt; dir 1 (B): target=yt, x=yp
    for d, (tlo, thi) in enumerate(((qyp_lo, qyp_hi), (qyt_lo, qyt_hi))):
        tlo32 = np.float32(tlo); thi32 = np.float32(thi)
        fbs = np.float32((thi32 - tlo32) / NB)
        ivf = np.float64(np.float32(1.0) / fbs)
        for ai, a in enumerate(ANCHORS):
            row[8 * d + ai] = np.float32(-SQPT * (ivf * tlo32 + a + 0.5))
        for z in CONV_Z:
            row[32 + 2 * z + d] = np.float32(
                -SQPT * (ivf * tlo32 + (4 * z + 2) + 0.5))
        for pi in range(NPAIR):
            ae = 2 * ZONE * pi
            row[16 + 4 * d + pi] = np.float32(-2.0 * PT * (ivf * tlo32 + ae + 0.5))
        row[24 + d] = np.float32(SQPT * ivf)
        row[26 + d] = np.float32(2.0 * PT * ivf)
    row[28] = np.float32(qyt_lo); row[29] = np.float32(qyt_hi)
    row[30] = np.float32(qyp_lo); row[31] = np.float32(qyp_hi)
    return np.ascontiguousarray(np.broadcast_to(row.reshape(1, -1), (P, NCC)),
                                dtype=np.float32)


def _assemble(res):
    """Fold per-core outputs -> S0[2,32], S1[2,32], SX[2], SX2[2] (f64,
    unnormalized chains; gamma applied here)."""
    acc = np.zeros((P, 32))
    s1 = [np.zeros((P, 4, 4, 64)) for _ in range(2)]
    s0 = [np.zeros((P, 512)) for _ in range(2)]
    for r in res:
        acc += np.asarray(r["acc"], dtype=np.float64).reshape(P, 32)
        for d in range(2):
            s1[d] += np.asarray(r[f"s1_{d}"], dtype=np.float64).reshape(P, 4, 4, 64)
            s0[d] += np.asarray(r[f"s0_{d}"], dtype=np.float64).reshape(P, 512)

    rho = np.exp(2.0 * PT * ZONE) * np.float64(np.float32(C_PAIR))
    S0 = np.zeros((2, NB)); S1 = np.zeros((2, NB))
    SX = np.zeros(2); SX2 = np.zeros(2)
    ii = np.arange(64)
    for d in range(2):
        # S1 from trace diagonals
        for k in range(NB):
            z, b = k // 4, k % 4
            rows = 64 * (z % 2) + ii
            S1[d, k] = s1[d][rows, z // 2, b, ii].sum()
        # anchor S0 from ACT accums
        for ai, a in enumerate(ANCHORS):
            S0[d, a] = acc[:, 8 * d + ai].sum()
        # chain S0 from one-hot folds (converted mid-bins from ACT accums)
        for k in CHAIN_BINS:
            z, o = k // 4, k % 4
            if o == 2 and z in CONV_Z:
                S0[d, k] = acc[:, 16 + 2 * z + d].sum()
            else:
                S0[d, k] = s0[d][32 * (z % 4) + 3 * (z // 4) + (o - 1), :].sum()
        SX[d] = s0[d][6, :].sum()
        SX2[d] = s0[d][38, :].sum()
        # gamma compensation for chain bins (and pair-rho for 2nd zones).
        # Converted zones: slot2 is exact (corr 1); slot3 is ONE step from
        # the exact slot2 -> corr = exp(-5 PT) (one rho for odd zones).
        for k in CHAIN_BINS:
            z = k // ZONE
            j = k - z * ZONE
            conv = z in CONV_Z
            if conv and j == 2:
                continue                   # exact, no correction
            if conv and j == 3:
                corr = np.exp(-5.0 * PT)
                if z % 2 == 1:
                    corr /= rho
            else:
                corr = np.exp(-PT * j * j)
                if z % 2 == 1:
                    corr /= rho ** j
            S0[d, k] *= corr
            S1[d, k] *= corr
    return S0, S1, SX, SX2


def _final(S0, S1, SX, SX2):
    out = 0.0
    for d in range(2):
        tm = SX[d] / N
        mi = S1[d] / (S0[d] + EPS)
        bgv = float((S0[d] * (mi - tm) ** 2).sum() / (S0[d].sum() + EPS))
        tv = (SX2[d] - N * tm * tm) / (N - 1)
        out += bgv / (tv + EPS)
    return -out / 2.0


def kernel(y_true, y_pred):
    yt = np.ascontiguousarray(np.asarray(y_true, dtype=np.float32).reshape(-1))
    yp = np.ascontiguousarray(np.asarray(y_pred, dtype=np.float32).reshape(-1))
    assert yt.size == N and yp.size == N
    yt_sh = yt.reshape(NCORES, P, F)
    yp_sh = yp.reshape(NCORES, P, F)

    qyt_lo, qyt_hi = np.quantile(yt, (0.01, 0.99))
    qyp_lo, qyp_hi = np.quantile(yp, (0.01, 0.99))

    cst = _cst(qyt_lo, qyt_hi, qyp_lo, qyp_hi)
    ncm = _get_nc()
    in_maps = [{"yt": yt_sh[c], "yp": yp_sh[c], "cst": cst}
               for c in range(NCORES)]
    r = _run(ncm, in_maps)
    S0, S1, SX, SX2 = _assemble(r.results)
    return np.array(_final(S0, S1, SX, SX2), dtype=np.float32)


# revision 50
# speedup vs baseline: 1.2345x; 1.2345x over previous
"""CorrRatio (symmetric correlation-ratio loss) on 8 Trainium2 NeuronCores.

Strategy
--------
Input: y_true, y_pred f32 (1,1,128,128,128) -> N = 2^21 voxels, sharded
contiguously across 8 cores as [128, 2048] f32 tiles (all reductions are
order-independent, so contiguous sharding is exact).

Quantiles are computed EXACTLY on the host (np.quantile) - no device
phase needed. One NEFF does the whole main pass:

Per direction (A: target=y_pred/x=y_true, B: swapped), bins live on the
normalized axis yn = (clip(t)-lo)*ivf in [0,32]:
 * 8 ANCHOR bins a=0,4,..,28: ACT Derivative_Erf(scale*tc+bias) -> w_a
   (bf16), with accum_out giving S0_a per-partition for free.
 * 4 E tiles (one per anchor PAIR ae=0,8,16,24): ACT Exp. E encodes
   exp(2PT*(yn-ae-.5)); pairs ae=0,8 get a DVE min-clamp (ACT Exp
   overflows to Inf above arg 88, measured). Second zone of each pair
   uses E2 = E*exp(-2PT*4) (one DVE tensor_scalar per pair) so chain
   ops stay plain 2x-mode tensor_tensor.
 * 24 CHAIN bins: w_{a+j} = w_{a+j-1} * E(2) on DVE bf16 (j<=3;
   unnormalized - host multiplies gamma_j = exp(-PT j^2) back).
 * S0 for chain bins + SX/SX2: one-hot PE folds, col-tiled 4-way
   (tile_position via out base partition 0/32/64/96).
 * S1 for ALL 32 bins: PE "trace" matmuls - lhsT = 64-col chunk of
   x (bf16), rhs = the 8 bins of a group x same 64 cols, accumulated
   over 32 chunks into PSUM [64,8,64]; the diagonal [i,(b,i)] holds
   sum(w_b * x) partials. Host extracts diagonals (f64). This removes
   all 64 per-bin w*x DVE multiplies.

Everything (PSUM S1/S0 partials, ACT accums) is DMA'd out raw; the host
does the small algebra in f64.
"""

import numpy as np

import concourse.bacc as bacc
import concourse.bass as bass
import concourse.mybir as mybir
import concourse.tile as tile
from concourse import bass_utils

F32 = mybir.dt.float32
BF16 = mybir.dt.bfloat16
ALU = mybir.AluOpType
ACTF = mybir.ActivationFunctionType

NB = 32
SR = 1.0 / 2.355
PT = 1.0 / (2.0 * SR * SR)          # normalized preterm = 2.355^2/2
SQPT = float(np.sqrt(PT))
EPS = float(np.finfo(np.float32).eps)
NCORES = 8
N = 128 * 128 * 128                  # 2097152 voxels
V = N // NCORES                      # 262144 per core
P = 128
F = V // P                           # 2048 free-dim per partition

ZONE = 4                             # bins per anchor
NA = NB // ZONE                      # 8 anchors/dir: 0,4,...,28
NPAIR = NA // 2                      # 4 E tiles/dir (anchor pairs)
C_PAIR = float(np.float32(np.exp(-2.0 * PT * ZONE)))   # E2 = E * C_PAIR
ANCHORS = [ZONE * i for i in range(NA)]
CHAIN_BINS = [8 * g + o for g in range(4) for o in (1, 2, 3, 5, 6, 7)]
ECLAMP = 3.0e38                      # min-clamp for Exp Inf overflow

CONV_Z = ()                # zones whose slot-2 bin is a
                                     # direct DErf (both dirs): ACT has
                                     # slack, DVE is the wall
NCC = 40                             # cst columns


def _build_main():
    nc = bacc.Bacc("TRN2", target_bir_lowering=False, debug=False,
                   num_devices=NCORES)
    yt_d = nc.dram_tensor("yt", [P, F], F32, kind="ExternalInput").ap()
    yp_d = nc.dram_tensor("yp", [P, F], F32, kind="ExternalInput").ap()
    cst_d = nc.dram_tensor("cst", [P, NCC], F32, kind="ExternalInput").ap()
    s1_d = [nc.dram_tensor(f"s1_{d}", [P, 4, 4, 64], F32,
                           kind="ExternalOutput").ap() for d in range(2)]
    s0_d = [nc.dram_tensor(f"s0_{d}", [P, 512], F32,
                           kind="ExternalOutput").ap() for d in range(2)]
    acc_d = nc.dram_tensor("acc", [P, 32], F32, kind="ExternalOutput").ap()

    with tile.TileContext(nc) as tc:
        with (
            tc.tile_pool(name="io", bufs=1) as io_pool,
            tc.tile_pool(name="clip", bufs=1) as clip_pool,
            tc.tile_pool(name="xp", bufs=1) as x_pool,
            tc.tile_pool(name="ep", bufs=3) as e_pool,
            tc.tile_pool(name="wp", bufs=2) as w_pool,
            tc.tile_pool(name="stat", bufs=1) as stat_pool,
            tc.tile_pool(name="psum", bufs=1, space="PSUM") as psum_pool,
        ):
            yt = io_pool.tile([P, F], F32)
            yp = io_pool.tile([P, F], F32)
            cst = io_pool.tile([P, NCC], F32)
            FH = F // 2
            nc.sync.dma_start(cst[:], cst_d)
            nc.sync.dma_start(yp[:], yp_d)
            nc.sync.dma_start(yt[:], yt_d)

            # Dummy 1-element Exp on cst: pulls the ~1.5us Exp table load
            # off the critical path (it otherwise waits for the first real
            # Exp's input clip at ~9us).
            warm = stat_pool.tile([P, 1], BF16, tag="warm", name="warm")
            nc.scalar.activation(warm[:], cst[:, 0:1], ACTF.Exp)

            # --- clips, IN-PLACE (elementwise same-index streaming is
            # read-ahead-of-write through the DVE pipe): frees 16KB/part,
            # spent on a deeper wz pool
            nc.vector.tensor_scalar(
                out=yp[:], in0=yp[:], scalar1=cst[:, 31:32],
                scalar2=cst[:, 30:31], op0=ALU.min, op1=ALU.max)
            nc.vector.tensor_scalar(
                out=yt[:], in0=yt[:], scalar1=cst[:, 29:30],
                scalar2=cst[:, 28:29], op0=ALU.min, op1=ALU.max)
            ypc, ytc = yp, yt
            xb = []                       # xb[d]: the "x" tensor of dir d
            for d, src in ((0, ytc), (1, ypc)):
                t = x_pool.tile([P, F], BF16, tag=f"xb{d}", name=f"xb{d}")
                nc.vector.tensor_scalar(out=t[:], in0=src[:], scalar1=1.0,
                                        scalar2=None, op0=ALU.mult)
                xb.append(t)
            xsq = []
            for d in range(2):
                t = x_pool.tile([P, F], BF16, tag=f"xsq{d}", name=f"xsq{d}")
                nc.scalar.activation(t[:], xb[d][:], ACTF.Square)
                xsq.append(t)

            # pre-scaled bin-domain inputs, IN-PLACE on the clipped tiles
            # (the xb casts above already consumed the unscaled values;
            # WAR is serialized by the framework). ACT AP-scale measures
            # ~0.5us extra per op, so scale once on DVE; DErf uses scale=1
            # and Exp the compile-time 2*SQPT.
            for d, src_t in ((0, ypc), (1, ytc)):
                nc.vector.tensor_scalar(out=src_t[:], in0=src_t[:],
                                        scalar1=cst[:, 24 + d:25 + d],
                                        scalar2=None, op0=ALU.mult)
            tcs = (ypc, ytc)              # target (binned) tensor per dir

            # --- sliding one-hot for S0 folds: Z[:, 31] = 1
            Z = stat_pool.tile([P, 63], BF16)
            nc.vector.memset(Z[:], 0.0)
            nc.vector.memset(Z[:, 31:32], 1.0)

            # --- accum tile for anchor S0 (ACT accum_out)
            acc = stat_pool.tile([P, 32], F32)

            # --- w-tile allocation + anchor DErf (hoistable so the first
            # zone-pair's anchors precede the Exp block: costs one extra
            # ACT table switch but removes ~15us of pipeline head).
            wz_tiles = {}

            def mk_tiles(q, d):
                pair = []
                for z in (2 * q, 2 * q + 1):
                    wz = w_pool.tile([P, 4, F], BF16, tag="wz", bufs=7,
                                     name=f"w{d}_{z}")
                    nc.scalar.activation(
                        wz[:, 0, :], tcs[d][:], ACTF.Derivative_Erf,
                        bias=cst[:, 8 * d + z:8 * d + z + 1],
                        scale=1.0,
                        accum_out=acc[:, 8 * d + z:8 * d + z + 1])
                    pair.append(wz)
                wz_tiles[(q, d)] = pair

            # --- E tiles. ACT order minimizes table switches while
            # letting the pipeline start early: Exp(pair0) -> DErf(q0)
            # -> Exp(pairs 1-3) -> DErf(q1..q3). The DVE parts (Inf-clamp
            # for pairs 0/1, E2 = E*C_PAIR) are deferred into the q-loop
            # so chains of earlier zones aren't queued behind them.
            eraw = {}

            def mk_exp(pi):
                for d in range(2):
                    t = e_pool.tile([P, F], BF16, tag="eraw", bufs=8,
                                    name=f"eraw{d}_{pi}")
                    nc.scalar.activation(
                        t[:], tcs[d][:], ACTF.Exp,
                        bias=cst[:, 16 + 4 * d + pi:17 + 4 * d + pi],
                        scale=2.0 * SQPT)
                    eraw[(d, pi)] = t

            e1 = {}
            e2 = {}

            def mk_epost(pi):
                for d in range(2):
                    r = eraw[(d, pi)]
                    if pi < 2:            # ae = 0, 8: Exp can hit Inf
                        t = e_pool.tile([P, F], BF16, tag="ecl", bufs=2,
                                        name=f"ecl{d}_{pi}")
                        nc.vector.tensor_scalar(out=t[:], in0=r[:],
                                                scalar1=ECLAMP, scalar2=None,
                                                op0=ALU.min)
                        e1[(d, pi)] = t
                    else:
                        e1[(d, pi)] = r
                    t2 = e_pool.tile([P, F], BF16, tag="e2", bufs=2,
                                     name=f"e2_{d}_{pi}")
                    nc.vector.tensor_scalar(out=t2[:], in0=e1[(d, pi)][:],
                                            scalar1=C_PAIR, scalar2=None,
                                            op0=ALU.mult)
                    e2[(d, pi)] = t2

            mk_exp(0)
            mk_tiles(0, 0)
            mk_tiles(0, 1)
            for pi in (1, 2, 3):
                mk_exp(pi)

            # --- PSUM tiles
            # ps1[d][64*(z%2)+i, z//2, b, i'] accumulates
            #   sum_c sum_p xb[p,64c+i] * w_{4z+b}[p,64c+i']  (diag i==i')
            ps1 = [psum_pool.tile([P, 4, 4, 64], F32, tag=f"ps1_{d}",
                                  name=f"ps1_{d}") for d in range(2)]
            ps0 = [psum_pool.tile([P, 512], F32, tag=f"ps0_{d}",
                                  name=f"ps0_{d}") for d in range(2)]


            # S0 fold bookkeeping: chain bin k=4z+o -> col-group j=z%4,
            # row 3*(z//4)+(o-1); SX -> (j=0,row 6), SX2 -> (j=1,row 6).
            # One PSUM accumulation group per (d, col-group): count MMs so
            # start/stop land on the first/last.
            nmm_j = []                    # per col-group MM counts
            for j in range(4):
                n = 4 if j < 2 else 0     # SX / SX2
                for z in (j, j + 4):
                    n += 4 * sum(1 for o in (1, 2, 3)
                                 if not (o == 2 and z in CONV_Z))
                nmm_j.append(n)
            mm_ctr = [[0] * 4 for _ in range(2)]

            def s0_fold_mm(d, j, row, rhs_tile, slot, c):
                rhs = (rhs_tile[:, slot, c * FC:(c + 1) * FC]
                       if slot is not None
                       else rhs_tile[:, c * FC:(c + 1) * FC])
                i = mm_ctr[d][j]
                nc.tensor.matmul(
                    ps0[d][32 * j:32 * j + 32, :],
                    Z[:, 31 - row:63 - row], rhs,
                    start=(i == 0), stop=(i == nmm_j[j] - 1),
                    tile_position=(0, 32 * j))
                mm_ctr[d][j] += 1

            # --- per direction, per zone PAIR (2 zones of anchor+3 chains).
            # Matmuls of the two zones interleave so adjacent MMs hit
            # disjoint PE col-groups (out base 0-63 vs 64-127 / col-group
            # j vs j+1) and stream concurrently.
            FC = F // 4                   # 512-col chunks for S0 folds
            CH = 64                       # 64-col chunks for S1 traces
            NCH = F // CH                 # 32
            # SX / SX2 folds first: their inputs are ready early, and
            # putting them first in the (d, col-group) accumulation groups
            # moves them off the kernel tail.
            for c in range(4):
                for d in range(2):
                    s0_fold_mm(d, 0, 6, xb[d], None, c)
                    s0_fold_mm(d, 1, 6, xsq[d], None, c)

            for q in range(4):
                mk_epost(q)
                # prefetch next pair's anchor DErfs on ACT so its chains
                # (DVE) aren't serialized behind this pair's PE/ACT work
                if q + 1 < 4:
                    mk_tiles(q + 1, 0)
                    mk_tiles(q + 1, 1)
                for d in range(2):
                    if (q, d) not in wz_tiles:
                        mk_tiles(q, d)
                    wzs = wz_tiles[(q, d)]
                    for zi, z in enumerate((2 * q, 2 * q + 1)):
                        et = e1[(d, q)] if z % 2 == 0 else e2[(d, q)]
                        conv = z in CONV_Z
                        nc.vector.tensor_tensor(
                            out=wzs[zi][:, 1, :], in0=wzs[zi][:, 0, :],
                            in1=et[:], op=ALU.mult)
                        if conv:
                            ci = 2 * z + d
                            nc.scalar.activation(
                                wzs[zi][:, 2, :], tcs[d][:],
                                ACTF.Derivative_Erf,
                                bias=cst[:, 32 + ci:33 + ci],
                                scale=1.0,
                                accum_out=acc[:, 16 + ci:17 + ci])
                        else:
                            nc.vector.tensor_tensor(
                                out=wzs[zi][:, 2, :], in0=wzs[zi][:, 1, :],
                                in1=et[:], op=ALU.mult)
                        nc.vector.tensor_tensor(
                            out=wzs[zi][:, 3, :], in0=wzs[zi][:, 2, :],
                            in1=et[:], op=ALU.mult)

                    # S1 trace matmuls, pair-interleaved over chunks
                    for c in range(NCH):
                        for zi, z in enumerate((2 * q, 2 * q + 1)):
                            nc.tensor.matmul(
                                ps1[d][64 * zi:64 * zi + 64, z // 2, :, :],
                                xb[d][:, c * CH:(c + 1) * CH],
                                wzs[zi][:, :, c * CH:(c + 1) * CH],
                                start=(c == 0), stop=(c == NCH - 1))

                    # S0 one-hot folds, pair-interleaved (adjacent j's)
                    for o in (1, 2, 3):
                        for c in range(4):
                            for zi, z in enumerate((2 * q, 2 * q + 1)):
                                if o == 2 and z in CONV_Z:
                                    continue
                                s0_fold_mm(d, z % 4, 3 * (z // 4) + (o - 1),
                                           wzs[zi], o, c)

                    # stage+DMA this pair's finished S1 PSUM slice (PSUM is
                    # not DMA-able; ACT Copy - it's in every table set and
                    # ScalarE sits closest to PSUM). Overlaps later pairs.
                    st = stat_pool.tile([P, 4, 64], F32, tag=f"s1s{d}",
                                        bufs=2, name=f"s1s{d}_{q}")
                    nc.scalar.activation(st[:], ps1[d][:, q, :, :], ACTF.Copy)
                    nc.sync.dma_start(s1_d[d][:, q, :, :], st[:])

            # --- DMA the rest out (host does the algebra)
            nc.sync.dma_start(acc_d, acc[:])
            for d in range(2):
                s0s = stat_pool.tile([P, 512], F32, tag=f"s0s{d}",
                                     name=f"s0s{d}")
                nc.scalar.activation(s0s[:], ps0[d][:], ACTF.Copy)
                nc.sync.dma_start(s0_d[d], s0s[:])
    nc.compile()
    return nc


_NC_CACHE = {}


def _get_nc(which="main"):
    if which not in _NC_CACHE:
        _NC_CACHE[which] = _build_main()
    return _NC_CACHE[which]


def _run(nc, in_maps, trace=False):
    return bass_utils.run_bass_kernel_spmd(
        nc, in_maps, core_ids=list(range(NCORES)), trace=trace)


def _cst(qyt_lo, qyt_hi, qyp_lo, qyp_hi):
    row = np.zeros(NCC, dtype=np.float32)
    # dir 0 (A): target=yp, x=yt; dir 1 (B): target=yt, x=yp
    for d, (tlo, thi) in enumerate(((qyp_lo, qyp_hi), (qyt_lo, qyt_hi))):
        tlo32 = np.float32(tlo); thi32 = np.float32(thi)
        fbs = np.float32((thi32 - tlo32) / NB)
        ivf = np.float64(np.float32(1.0) / fbs)
        for ai, a in enumerate(ANCHORS):
            row[8 * d + ai] = np.float32(-SQPT * (ivf * tlo32 + a + 0.5))
        for z in CONV_Z:
            row[32 + 2 * z + d] = np.float32(
                -SQPT * (ivf * tlo32 + (4 * z + 2) + 0.5))
        for pi in range(NPAIR):
            ae = 2 * ZONE * pi
            row[16 + 4 * d + pi] = np.float32(-2.0 * PT * (ivf * tlo32 + ae + 0.5))
        row[24 + d] = np.float32(SQPT * ivf)
        row[26 + d] = np.float32(2.0 * PT * ivf)
    row[28] = np.float32(qyt_lo); row[29] = np.float32(qyt_hi)
    row[30] = np.float32(qyp_lo); row[31] = np.float32(qyp_hi)
    return np.ascontiguousarray(np.broadcast_to(row.reshape(1, -1), (P, NCC)),
                                dtype=np.float32)


def _assemble(res):
    """Fold per-core outputs -> S0[2,32], S1[2,32], SX[2], SX2[2] (f64,
    unnormalized chains; gamma applied here)."""
    acc = np.zeros((P, 32))
    s1 = [np.zeros((P, 4, 4, 64)) for _ in range(2)]
    s0 = [np.zeros((P, 512)) for _ in range(2)]
    for r in res:
        acc += np.asarray(r["acc"], dtype=np.float64).reshape(P, 32)
        for d in range(2):
            s1[d] += np.asarray(r[f"s1_{d}"], dtype=np.float64).reshape(P, 4, 4, 64)
            s0[d] += np.asarray(r[f"s0_{d}"], dtype=np.float64).reshape(P, 512)

    rho = np.exp(2.0 * PT * ZONE) * np.float64(np.float32(C_PAIR))
    S0 = np.zeros((2, NB)); S1 = np.zeros((2, NB))
    SX = np.zeros(2); SX2 = np.zeros(2)
    ii = np.arange(64)
    for d in range(2):
        # S1 from trace diagonals
        for k in range(NB):
            z, b = k // 4, k % 4
            rows = 64 * (z % 2) + ii
            S1[d, k] = s1[d][rows, z // 2, b, ii].sum()
        # anchor S0 from ACT accums
        for ai, a in enumerate(ANCHORS):
            S0[d, a] = acc[:, 8 * d + ai].sum()
        # chain S0 from one-hot folds (converted mid-bins from ACT accums)
        for k in CHAIN_BINS:
            z, o = k // 4, k % 4
            if o == 2 and z in CONV_Z:
                S0[d, k] = acc[:, 16 + 2 * z + d].sum()
            else:
                S0[d, k] = s0[d][32 * (z % 4) + 3 * (z // 4) + (o - 1), :].sum()
        SX[d] = s0[d][6, :].sum()
        SX2[d] = s0[d][38, :].sum()
        # gamma compensation for chain bins (and pair-rho for 2nd zones).
        # Converted zones: slot2 is exact (corr 1); slot3 is ONE step from
        # the exact slot2 -> corr = exp(-5 PT) (one rho for odd zones).
        for k in CHAIN_BINS:
            z = k // ZONE
            j = k - z * ZONE
            conv = z in CONV_Z
            if conv and j == 2:
                continue                   # exact, no correction
            if conv and j == 3:
                corr = np.exp(-5.0 * PT)
                if z % 2 == 1:
                    corr /= rho
            else:
                corr = np.exp(-PT * j * j)
                if z % 2 == 1:
                    corr /= rho ** j
            S0[d, k] *= corr
            S1[d, k] *= corr
    return S0, S1, SX, SX2


def _final(S0, S1, SX, SX2):
    out = 0.0
    for d in range(2):
        tm = SX[d] / N
        mi = S1[d] / (S0[d] + EPS)
        bgv = float((S0[d] * (mi - tm) ** 2).sum() / (S0[d].sum() + EPS))
        tv = (SX2[d] - N * tm * tm) / (N - 1)
        out += bgv / (tv + EPS)
    return -out / 2.0


def kernel(y_true, y_pred):
    yt = np.ascontiguousarray(np.asarray(y_true, dtype=np.float32).reshape(-1))
    yp = np.ascontiguousarray(np.asarray(y_pred, dtype=np.float32).reshape(-1))
    assert yt.size == N and yp.size == N
    yt_sh = yt.reshape(NCORES, P, F)
    yp_sh = yp.reshape(NCORES, P, F)

    qyt_lo, qyt_hi = np.quantile(yt, (0.01, 0.99))
    qyp_lo, qyp_hi = np.quantile(yp, (0.01, 0.99))

    cst = _cst(qyt_lo, qyt_hi, qyp_lo, qyp_hi)
    ncm = _get_nc()
    in_maps = [{"yt": yt_sh[c], "yp": yp_sh[c], "cst": cst}
               for c in range(NCORES)]
    r = _run(ncm, in_maps)
    S0, S1, SX, SX2 = _assemble(r.results)
    return np.array(_final(S0, S1, SX, SX2), dtype=np.float32)
